# revision 1
# baseline (speedup 1.0000x reference)
"""MetaSR (meta-upscale CNN) Trainium2 kernel, SPMD over 8 NeuronCores.

Algorithm (bilinear reformulation of the reference):
    feat = relu(conv5x5(x) + b)                      [N,64,H,W]
    hid  = relu(pos @ w1 + b1)                       [(H*s*W*s), 256]
    out[n,p,l,c] = sum_h hid[r(p,l),h] * U[n,l,h,c] + bias[n,l,c] + mean_c
      where U[n,l,h,c] = sum_k cols[n,l,k] * w2[h, k*3+c]   (k = 3x3 unfold taps x 64 ch)
            bias[n,l,c] = sum_k cols[n,l,k] * b2[k*3+c]
This avoids materializing the per-pixel weight tensor lw = hid@w2 (452MB) and
cuts the per-pixel contraction from 576 to 256 terms.

Sharding: 8 horizontal strips of 16 image rows each (all of N on every core).

v2 pipeline per core:
  - conv as im2col matmul (K=75) -> feat [64, 18*130] fp32r; two shifted
    duplicates into partitions 64..127 (+1 col in `feat`, +1 row in `featb`)
    pack the nine 3x3 taps into 5 K=128 matmul blocks.
  - stage B (PE, fp32r): U^T psum tiles [128 h-chunk, 512 pix] = w2r block^T @
    feat-window; 6 m-blocks (3 ch x 2 h-chunks) + bias block [4, 512].
  - ACT evicts U^T psum -> SBUF fp16; DVE multiplies with hid^T fp16
    (MLP hidden states, also [h, pix] layout); PE reduces over h via
    ones-vector fp16 matmuls, col-group packed (tile_position) so the four
    subpixel outputs land on psum partitions {0,32,64,96}; a 3rd tiny matmul
    adds the per-pixel bias + channel mean from an SBUF row.
"""
import os
import numpy as np

SCALE = 2
RGB_MEAN = (0.4488, 0.4371, 0.404)
N, C, H, W = 4, 3, 128, 128
G0 = 64
NCORES = 8
HS = H // NCORES          # image rows per core (16)
FR = HS + 2               # feat rows incl unfold halo (18)
FC = W + 2                # feat cols incl unfold halo (130)
FREE = FR * FC            # 2340
HH = 256                  # MLP hidden
WCOLS = 3 * HH + 4        # 772 = (c,h) cols + 3 bias cols + 1 pad
LP = HS * W               # pixels per core (2048)
PR = 4 * LP               # pos rows per core (8192)
NLT = LP // 512           # 512-pixel tiles per core (4)

# tap blocks: (weight-block index) -> (which feat tile, base tap)
# blocks 0-2: tap pairs (t,t+1) via the +1-col dup; block 3: pair (t2,t5)
# via the +1-row dup; block 4: single tap t8 (upper weight rows zero).
BLK = [("f", 0), ("f", 3), ("f", 6), ("b", 2), ("f", 8)]

_CACHE = {}


def _build_nc(mmdt_name="float16", reps=1):
    import concourse.bass as bass
    import concourse.tile as tile
    from concourse import bacc, mybir
    from contextlib import nullcontext

    mmdt = getattr(mybir.dt, mmdt_name)
    f32 = mybir.dt.float32
    f16 = mybir.dt.float16
    zdt = mybir.dt.uint16 if mybir.dt.size(mmdt) == 2 else f32

    nc = bacc.Bacc("TRN2", target_bir_lowering=False, debug=False,
                   num_devices=NCORES)

    xs = nc.dram_tensor("xs", [N, C, HS + 6, W + 6], mmdt, kind="ExternalInput").ap()
    posT = nc.dram_tensor("posT", [4, PR], mmdt, kind="ExternalInput").ap()
    cwr = nc.dram_tensor("cwr", [75, G0], mmdt, kind="ExternalInput").ap()
    cb = nc.dram_tensor("cb", [G0, 1], f32, kind="ExternalInput").ap()
    w1a = nc.dram_tensor("w1a", [4, HH], mmdt, kind="ExternalInput").ap()
    w2r = nc.dram_tensor("w2r", [5, 128, WCOLS], mmdt, kind="ExternalInput").ap()
    fmask = nc.dram_tensor("fmask", [G0, FREE], mmdt, kind="ExternalInput").ap()
    mean4 = nc.dram_tensor("mean4", [4, 1], f32, kind="ExternalInput").ap()
    ones16 = nc.dram_tensor("ones16", [128, 128], f16, kind="ExternalInput").ap()
    out = nc.dram_tensor("out", [N, 3, 4, LP], f32, kind="ExternalOutput").ap()

    XW6 = W + 6  # 134

    with tile.TileContext(nc) as tc:
        with tc.tile_pool(name="const", bufs=1) as cpool, \
             tc.tile_pool(name="feat", bufs=1) as fpool, \
             tc.tile_pool(name="hid", bufs=1) as hpool, \
             tc.tile_pool(name="im2col", bufs=2) as xpool, \
             tc.tile_pool(name="usb", bufs=4) as upool, \
             tc.tile_pool(name="pt", bufs=3) as ppool, \
             tc.tile_pool(name="bm", bufs=2) as bmpool, \
             tc.tile_pool(name="ups", bufs=2, space="PSUM") as ups, \
             tc.tile_pool(name="biasps", bufs=2, space="PSUM") as biasps, \
             tc.tile_pool(name="outps", bufs=2, space="PSUM") as outps:

            # ---- constants to SBUF ----
            cwr_t = cpool.tile([75, G0], mmdt, tag="cwr")
            nc.sync.dma_start(cwr_t[:], cwr[:])
            cb_t = cpool.tile([G0, 1], f32, tag="cb")
            nc.sync.dma_start(cb_t[:], cb[:])
            w1a_t = cpool.tile([4, HH], mmdt, tag="w1a")
            nc.sync.dma_start(w1a_t[:], w1a[:])
            fmask_t = cpool.tile([G0, FREE], mmdt, tag="fmask")
            nc.sync.dma_start(fmask_t[:], fmask[:])
            mean4_t = cpool.tile([4, 1], f32, tag="mean4")
            nc.sync.dma_start(mean4_t[:], mean4[:])
            ones_t = cpool.tile([128, 128], f16, tag="ones16")
            nc.sync.dma_start(ones_t[:], ones16[:])
            w2r_t = []
            for b in range(5):
                t = cpool.tile([128, WCOLS], mmdt, tag=f"w2r{b}")
                nc.sync.dma_start(t[:], w2r[b])
                w2r_t.append(t)

            loop_ctx = tc.For_i(0, reps, 1, staggered_reset=True,
                                  hint_engines=(mybir.EngineType.PE,
                                                mybir.EngineType.DVE,
                                                mybir.EngineType.Activation)) \
                if reps > 1 else nullcontext()
            with loop_ctx:
              # ---- conv5x5 + relu -> feat strips + shifted duplicates ----
              feat, featb = [], []
              for n in range(N):
                  ft = fpool.tile([128, FREE], mmdt, tag=f"feat{n}")
                  fb = fpool.tile([128, FREE], mmdt, tag=f"featb{n}")
                  xt = xpool.tile([75, FREE], mmdt, tag="x")
                  for c in range(C):
                      for di in range(5):
                          src = bass.AP(xs.tensor,
                                        (n * C + c) * (HS + 6) * XW6 + di * XW6,
                                        [[1, 5], [XW6, FR], [1, FC]])
                          dst = xt[c * 25 + di * 5: c * 25 + di * 5 + 5, :] \
                              .rearrange("p (r q) -> p r q", q=FC)
                          nc.sync.dma_start(dst, src)
                  for ch in range(5):
                      lo = ch * 512
                      hi = min(FREE, lo + 512)
                      ps = ups.tile([G0, 512], f32, tag="pu")
                      nc.tensor.matmul(ps[:, : hi - lo], cwr_t[:], xt[:, lo:hi],
                                       start=True, stop=True)
                      nc.scalar.activation(ft[0:G0, lo:hi], ps[:, : hi - lo],
                                           mybir.ActivationFunctionType.Relu,
                                           bias=cb_t[:], scale=1.0)
                  # zero out-of-image halo rows/cols (core-specific mask)
                  nc.vector.tensor_mul(ft[0:G0, :], ft[0:G0, :], fmask_t[:])
                  # dup: upper feat = +1 col, featb = base / +1 row
                  nc.sync.dma_start(ft[G0:128, 0:FREE - 1], ft[0:G0, 1:FREE])
                  nc.vector.memset(ft[G0:128, FREE - 1:FREE].bitcast(zdt), 0)
                  nc.sync.dma_start(fb[0:G0, :], ft[0:G0, :])
                  nc.sync.dma_start(fb[G0:128, 0:FREE - FC], ft[0:G0, FC:FREE])
                  nc.vector.memset(fb[G0:128, FREE - FC:FREE].bitcast(zdt), 0)
                  feat.append(ft)
                  featb.append(fb)

              # ---- MLP layer 1 -> hidT fp16 tiles [128 h, 4096 pix] ----
              # posT column order (host): lp*4096 + p*1024 + (l % 1024)
              hidT = [[None] * 2, [None] * 2]
              for hch in range(2):
                  for lp in range(2):
                      hb = hpool.tile([128, 4096], f16, tag=f"hid{hch}_{lp}")
                      for sub in range(8):
                          base = lp * 4096 + sub * 512
                          pchunk = xpool.tile([4, 512], mmdt, tag="posc")
                          nc.sync.dma_start(pchunk[:], posT[:, base:base + 512])
                          ps = ups.tile([128, 512], f32, tag="pu")
                          nc.tensor.matmul(ps[:],
                                           w1a_t[:, hch * 128:(hch + 1) * 128],
                                           pchunk[:],
                                           start=True, stop=True)
                          nc.scalar.activation(
                              hb[:, sub * 512:(sub + 1) * 512], ps[:],
                              mybir.ActivationFunctionType.Relu,
                              bias=0.0, scale=1.0)
                      hidT[hch][lp] = hb

              def win(n, lp, t, which, rows, half=0):
                  # window covering `rows` feat rows starting at image row
                  # lp*8 + half*4, shifted by tap t
                  ti, tj = t // 3, t % 3
                  srct = feat[n] if which == "f" else featb[n]
                  v = srct[:].rearrange("p (r q) -> p r q", q=FC)
                  r0 = lp * 8 + half * 4 + ti
                  return v[:, r0: r0 + rows, tj: tj + W]

              # ---- main loop: lp = 1024-pixel (8-row) tile ----
              for n in range(N):
                  for lp in range(2):
                      # bias psum [4, 512] x2 halves -> bm [1, 3072] fp16
                      bm = bmpool.tile([1, 3072], f16, tag="bm")
                      for half in range(2):
                          pb = biasps.tile([4, 512], f32, tag="pb")
                          for b in range(5):
                              which, t = BLK[b]
                              nc.tensor.matmul(pb[:], w2r_t[b][:, 768:772],
                                               win(n, lp, t, which, 4, half),
                                               start=(b == 0), stop=(b == 4))
                          bs = bmpool.tile([4, 512], f16, tag="bs")
                          nc.vector.tensor_scalar_add(bs[:], pb[:], mean4_t[:])
                          for cc in range(3):
                              nc.sync.dma_start(
                                  bm[0:1, cc * 1024 + half * 512:
                                     cc * 1024 + half * 512 + 512],
                                  bs[cc:cc + 1, :])

                      for cc in range(3):
                          pts = []
                          for hch in range(2):
                              mb = cc * 2 + hch
                              pu = ups.tile([128, 1024], f32, tag="pu")
                              for b in range(5):
                                  which, t = BLK[b]
                                  for hf in range(2):
                                      nc.tensor.matmul(
                                          pu[:, hf * 512:(hf + 1) * 512],
                                          w2r_t[b][:, mb * 128:(mb + 1) * 128],
                                          win(n, lp, t, which, 4, hf),
                                          start=(b == 0), stop=(b == 4))
                              us = upool.tile([128, 1024], f16, tag="us")
                              nc.scalar.activation(
                                  us[:], pu[:],
                                  mybir.ActivationFunctionType.Copy)
                              pt = ppool.tile([128, 4096], f16, tag="pt")
                              nc.vector.tensor_mul(
                                  pt[:].rearrange("p (a q) -> p a q", q=1024),
                                  us[:].unsqueeze(1).broadcast_to((128, 4, 1024)),
                                  hidT[hch][lp][:].rearrange(
                                      "p (a q) -> p a q", q=1024))
                              pts.append(pt)
                          for half in range(2):
                              po = outps.tile([128, 512], f32, tag="po")
                              for hch in range(2):
                                  for p in range(4):
                                      sl = slice(p * 1024 + half * 512,
                                                 p * 1024 + half * 512 + 512)
                                      nc.tensor.matmul(
                                          po[32 * p:32 * p + 32, :],
                                          ones_t[:, 0:32], pts[hch][:, sl],
                                          start=(hch == 0), stop=False,
                                          skip_group_check=True,
                                          tile_position=(0, 32 * p))
                              nc.tensor.matmul(
                                  po[:], ones_t[0:1, 0:128],
                                  bm[:, cc * 1024 + half * 512:
                                     cc * 1024 + half * 512 + 512],
                                  start=False, stop=True,
                                  skip_group_check=True)
                              posb = bmpool.tile([128, 512], f32, tag="posb")
                              nc.scalar.activation(
                                  posb[:], po[:],
                                  mybir.ActivationFunctionType.Copy)
                              posrc = posb[:].rearrange(
                                  "(a b) q -> a b q", b=32)[:, 0, :]
                              nc.sync.dma_start(
                                  out[n, cc][:, lp * 1024 + half * 512:
                                             lp * 1024 + half * 512 + 512],
                                  posrc)

    nc.compile()
    return nc


def _host_prep(x, pos_mat, conv_w, conv_b, w1, b1, w2, b2):
    f = np.float32
    mf = np.float16 if os.environ.get("MMDT", "float16") == "float16" else np.float32
    xpad = np.pad(x, ((0, 0), (0, 0), (3, 3), (3, 3))).astype(f)
    cwr = np.ascontiguousarray(conv_w.transpose(1, 2, 3, 0).reshape(75, G0)).astype(f)
    cb = conv_b.reshape(G0, 1).astype(f)
    w1a = np.vstack([w1, b1[None, :]]).astype(f)

    Wr = w2.reshape(HH, 576, 3)
    b2r = b2.reshape(576, 3)

    def tap_rows(t):
        return np.concatenate(
            [np.ascontiguousarray(Wr[:, t::9, :].transpose(1, 2, 0)).reshape(G0, 768),
             b2r[t::9, :], np.zeros((G0, 1), f)], axis=1)

    blocks = []
    for pair in ((0, 1), (3, 4), (6, 7), (2, 5)):
        blocks.append(np.vstack([tap_rows(pair[0]), tap_rows(pair[1])]))
    blocks.append(np.vstack([tap_rows(8), np.zeros((G0, WCOLS), f)]))
    w2rb = np.stack(blocks).astype(f)

    mean4 = np.zeros((4, 1), f)
    mean4[:3, 0] = np.asarray(RGB_MEAN, f) * 255.0
    ones16 = np.ones((128, 128), np.float16)

    in_maps = []
    for core in range(NCORES):
        xsl = np.ascontiguousarray(xpad[:, :, HS * core: HS * core + HS + 6, :])
        pos = pos_mat[0, PR * core: PR * (core + 1), :]
        pos = pos.reshape(2, 8, 2, W, 2, 3).transpose(0, 2, 4, 1, 3, 5).reshape(PR, 3)
        posTc = np.ascontiguousarray(
            np.concatenate([pos, np.ones((PR, 1), f)], 1).T).astype(f)
        fm = np.ones((FR, FC), f)
        fm[:, 0] = 0.0
        fm[:, FC - 1] = 0.0
        if core == 0:
            fm[0, :] = 0.0
        if core == NCORES - 1:
            fm[FR - 1, :] = 0.0
        in_maps.append({"xs": xsl.astype(mf), "posT": posTc.astype(mf),
                        "cwr": cwr.astype(mf), "cb": cb,
                        "w1a": w1a.astype(mf), "w2r": w2rb.astype(mf),
                        "fmask": np.broadcast_to(fm.reshape(1, FREE),
                                                 (G0, FREE)).astype(mf),
                        "mean4": mean4, "ones16": ones16})
    return in_maps


def _assemble(results):
    full = np.empty((N, 3, H * SCALE, W * SCALE), np.float32)
    for core in range(NCORES):
        r = results[core]["out"].reshape(N, 3, 2, 2, HS, W)
        blk = r.transpose(0, 1, 4, 2, 5, 3).reshape(N, 3, HS * 2, W * 2)
        full[:, :, HS * 2 * core: HS * 2 * (core + 1), :] = blk
    return full


def kernel(**inputs):
    from concourse.bass_utils import run_bass_kernel_spmd
    if "nc" not in _CACHE:
        _CACHE["nc"] = _build_nc(os.environ.get("MMDT", "float16"))
    in_maps = _host_prep(**inputs)
    res = run_bass_kernel_spmd(_CACHE["nc"], in_maps, list(range(NCORES)))
    _CACHE["last_result"] = res
    return _assemble(res.results)



# revision 10
# speedup vs baseline: 1.1508x; 1.1508x over previous
"""MetaSR (meta-upscale CNN) Trainium2 kernel, SPMD over 8 NeuronCores.

Algorithm (bilinear reformulation of the reference):
    feat = relu(conv5x5(x) + b)                      [N,64,H,W]
    hid  = relu(pos @ w1 + b1)                       [(H*s*W*s), 256]
    out[n,p,l,c] = sum_h hid[r(p,l),h] * U[n,l,h,c] + bias[n,l,c] + mean_c
      where U[n,l,h,c] = sum_k cols[n,l,k] * w2[h, k*3+c]   (k = 3x3 taps x 64)
            bias[n,l,c] = sum_k cols[n,l,k] * b2[k*3+c]

Sharding: 8 horizontal strips of 16 image rows each (all of N on every core).

v3 pipeline per core:
  - conv as im2col matmul; im2col built on HOST (incl. a -1e4 halo-mask row
    so no on-device fmask multiply), one DMA per n. ACT evicts relu directly
    to fp8e4 (scale 8) into ftb [128, 2*FREE]: cols [0:FREE] = (base | +1col),
    cols [FREE:2FREE] = (base | +1row) via 3 SBUF-SBUF dup DMAs per n.
  - MLP1 from a single posT [4,8192] SBUF tile, interleaved with conv.
  - stage B in fp8e4 DoubleRow: K=576(+pad) as 2 DR matmuls (4 k-tiles) +
    1 plain fp8 matmul, w2 prescaled x16; ACT evicts psum/128 -> us fp16.
  - pt = us (bcast x4) * hidT on DVE (4/6) and GpSimd (2/6).
  - PE reduces over h via ones fp16 matmuls col-group packed (subpixels on
    psum partitions {0,32,64,96}); bias+mean injected via a K=4 sel matmul
    from bs = pb/128 + mean (ACT). Output DMA'd straight from PSUM.
  - reduce(cc) is emitted one stage-B step late so DVE/Pool overlap PE.
"""
import os
import numpy as np

SCALE = 2
RGB_MEAN = (0.4488, 0.4371, 0.404)
N, C, H, W = 4, 3, 128, 128
G0 = 64
NCORES = 8
HS = H // NCORES          # image rows per core (16)
FR = HS + 2               # feat rows incl unfold halo (18)
FC = W + 2                # feat cols incl unfold halo (130)
FREE = FR * FC            # 2340
HH = 256                  # MLP hidden
WCOLS = 3 * HH + 4        # 772 = (c,h) cols + 3 bias cols + 1 pad
KIM = 76                  # im2col rows: 75 conv taps + halo-mask row
LP = HS * W               # pixels per core (2048)
PR = 4 * LP               # pos rows per core (8192)

FSCALE = 8.0              # feat fp8 scale
WSCALE = 16.0             # w2 fp8 scale
USCALE = 1.0 / (FSCALE * WSCALE)

_CACHE = {}


def _build_nc():
    import concourse.bass as bass
    import concourse.tile as tile
    from concourse import bacc, mybir

    f32 = mybir.dt.float32
    f16 = mybir.dt.float16
    f8 = mybir.dt.float8e4
    DR = mybir.MatmulPerfMode.DoubleRow

    nc = bacc.Bacc("TRN2", target_bir_lowering=False, debug=False,
                   num_devices=NCORES)

    xcol = nc.dram_tensor("xcol", [KIM, N * FREE], f16, kind="ExternalInput").ap()
    posT = nc.dram_tensor("posT", [4, PR], f16, kind="ExternalInput").ap()
    cwr2 = nc.dram_tensor("cwr2", [KIM, G0], f16, kind="ExternalInput").ap()
    cb8 = nc.dram_tensor("cb8", [G0, 1], f32, kind="ExternalInput").ap()
    w1a = nc.dram_tensor("w1a", [4, HH], f16, kind="ExternalInput").ap()
    w2dr = nc.dram_tensor("w2dr", [2, 128, 1600], f8, kind="ExternalInput").ap()
    w2s = nc.dram_tensor("w2s", [128, 800], f8, kind="ExternalInput").ap()
    sel = nc.dram_tensor("sel", [4, 384], f16, kind="ExternalInput").ap()
    mean4 = nc.dram_tensor("mean4", [4, 1], f32, kind="ExternalInput").ap()
    ones16 = nc.dram_tensor("ones16", [128, 32], f16, kind="ExternalInput").ap()
    out = nc.dram_tensor("out", [N, 3, 4, LP], f32, kind="ExternalOutput").ap()

    with tile.TileContext(nc) as tc:
        with tc.tile_pool(name="const", bufs=1) as cpool, \
             tc.tile_pool(name="feat", bufs=1) as fpool, \
             tc.tile_pool(name="hid", bufs=1) as hpool, \
             tc.tile_pool(name="im2col", bufs=2) as xpool, \
             tc.tile_pool(name="usb", bufs=3) as upool, \
             tc.tile_pool(name="pt", bufs=4) as ppool, \
             tc.tile_pool(name="bsb", bufs=2) as bpool, \
             tc.tile_pool(name="ups", bufs=2, space="PSUM") as ups, \
             tc.tile_pool(name="biasps", bufs=1, space="PSUM") as biasps, \
             tc.tile_pool(name="outps", bufs=2, space="PSUM") as outps:

            # ---- constants to SBUF; conv-critical ones first on sync ----
            cwr2_t = cpool.tile([KIM, G0], f16, tag="cwr2")
            nc.sync.dma_start(cwr2_t[:], cwr2[:])
            cb8_t = cpool.tile([G0, 1], f32, tag="cb8")
            nc.sync.dma_start(cb8_t[:], cb8[:])
            xt0 = xpool.tile([KIM, FREE], f16, tag="x")
            nc.sync.dma_start(
                xt0[:], bass.AP(xcol.tensor, 0, [[N * FREE, KIM], [1, FREE]]))
            # off-critical consts on other queues
            posT_t = cpool.tile([4, PR], f16, tag="posT")
            nc.gpsimd.dma_start(posT_t[:], posT[:])
            w1a_t = cpool.tile([4, HH], f16, tag="w1a")
            nc.gpsimd.dma_start(w1a_t[:], w1a[:])
            w2dr_t = []
            for p in range(2):
                t = cpool.tile([128, 1600], f8, tag=f"w2dr{p}")
                nc.scalar.dma_start(t[:], w2dr[p])
                w2dr_t.append(t)
            w2s_t = cpool.tile([128, 800], f8, tag="w2s")
            nc.scalar.dma_start(w2s_t[:], w2s[:])
            sel_t = cpool.tile([4, 384], f16, tag="sel")
            nc.scalar.dma_start(sel_t[:], sel[:])
            mean4_t = cpool.tile([4, 1], f32, tag="mean4")
            nc.scalar.dma_start(mean4_t[:], mean4[:])
            ones_t = cpool.tile([128, 32], f16, tag="ones16")
            nc.scalar.dma_start(ones_t[:], ones16[:])

            ftb = [fpool.tile([128, 2 * FREE], f8, tag=f"ftb{n}",
                              name=f"ftb{n}")
                   for n in range(N)]
            hidT = [[None] * 2, [None] * 2]

            def conv_n(n, xt):
                ft = ftb[n]
                for ch in range(5):
                    lo = ch * 512
                    hi = min(FREE, lo + 512)
                    ps = outps.tile([128, 512], f32, tag="po")
                    nc.tensor.matmul(ps[0:G0, : hi - lo], cwr2_t[:],
                                     xt[:, lo:hi], start=True, stop=True)
                    nc.scalar.activation(ft[0:G0, lo:hi], ps[0:G0, : hi - lo],
                                         mybir.ActivationFunctionType.Relu,
                                         bias=cb8_t[:], scale=FSCALE)
                u8 = mybir.dt.uint8
                # upper half of cols [0:FREE] = +1 element (col) shift
                nc.scalar.dma_start(ft[G0:128, 0:FREE - 1], ft[0:G0, 1:FREE])
                nc.vector.memset(ft[G0:128, FREE - 1:FREE].bitcast(u8), 0)
                # cols [FREE:2FREE] = (base | +1 row)
                nc.gpsimd.dma_start(ft[0:G0, FREE:2 * FREE], ft[0:G0, 0:FREE])
                nc.gpsimd.dma_start(ft[G0:128, FREE:2 * FREE - FC],
                                    ft[0:G0, FC:FREE])
                nc.vector.memset(
                    ft[G0:128, 2 * FREE - FC:2 * FREE].bitcast(u8), 0)

            def mlp1_tile(hch, lp):
                hb = hpool.tile([128, 4096], f16, tag=f"hid{hch}_{lp}")
                for pair in range(4):
                    ps = ups.tile([128, 1024], f32, tag="pu")
                    for sub in range(2):
                        base = lp * 4096 + pair * 1024 + sub * 512
                        nc.tensor.matmul(ps[:, sub * 512:(sub + 1) * 512],
                                         w1a_t[:, hch * 128:(hch + 1) * 128],
                                         posT_t[:, base:base + 512],
                                         start=True, stop=True)
                    nc.scalar.activation(
                        hb[:, pair * 1024:(pair + 1) * 1024], ps[:],
                        mybir.ActivationFunctionType.Relu, bias=0.0, scale=1.0)
                hidT[hch][lp] = hb

            # ---- conv + MLP1 interleaved ----
            mlp_order = [(0, 0), (0, 1), (1, 0), (1, 1)]
            for n in range(N):
                xt = xt0 if n == 0 else xpool.tile([KIM, FREE], f16, tag="x")
                if n > 0:
                    nc.sync.dma_start(
                        xt[:], bass.AP(xcol.tensor, n * FREE,
                                       [[N * FREE, KIM], [1, FREE]]))
                conv_n(n, xt)
                mlp1_tile(*mlp_order[n])

            # window AP into ftb[n]: k-tile pair (DoubleRow rhs) or single
            def win_dr(n, r0, off0, delta):
                ap = ftb[n][:]
                return bass.AP(ap.tensor, ap.offset + r0 * FC + off0,
                               [[2 * FREE, 128], [delta, 2], [FC, 4], [1, W]])

            def win_s(n, r0, off0):
                ap = ftb[n][:]
                return bass.AP(ap.tensor, ap.offset + r0 * FC + off0,
                               [[2 * FREE, 128], [FC, 4], [1, W]])

            D0 = FC                    # pair0: taps(0,1)@(r0,0) / (3,4)@(r0+1,0)
            O1 = 2 * FC                # pair1 ktile0: taps(6,7)@(r0+2,0)
            D1 = FREE - 2 * FC + 2     # pair1 ktile1: taps(2,5)@fb(r0,2)
            O2 = 2 * FC + 2            # single: tap8@(r0+2,2)

            def stage_b(n, lp, cc, hch, ptidx):
                mb = cc * 2 + hch
                pu = ups.tile([128, 1024], f32, tag="pu")
                for hf in range(2):
                    r0 = lp * 8 + hf * 4
                    sl = slice(hf * 512, (hf + 1) * 512)
                    lhs0 = w2dr_t[0][:, mb * 256:(mb + 1) * 256].rearrange(
                        "p (t m) -> p t m", t=2)
                    lhs1 = w2dr_t[1][:, mb * 256:(mb + 1) * 256].rearrange(
                        "p (t m) -> p t m", t=2)
                    nc.tensor.matmul(pu[:, sl], lhs0, win_dr(n, r0, 0, D0),
                                     start=True, stop=False, perf_mode=DR)
                    nc.tensor.matmul(pu[:, sl], lhs1, win_dr(n, r0, O1, D1),
                                     start=False, stop=False, perf_mode=DR)
                    nc.tensor.matmul(pu[:, sl],
                                     w2s_t[:, mb * 128:(mb + 1) * 128],
                                     win_s(n, r0, O2),
                                     start=False, stop=True)
                us = upool.tile([128, 1024], f16, tag="us")
                nc.scalar.activation(us[:], pu[:],
                                     mybir.ActivationFunctionType.Copy,
                                     bias=0.0, scale=USCALE)
                pt = ppool.tile([128, 4096], f16, tag="pt")
                eng = nc.gpsimd if ptidx in (0, 3) else nc.vector
                eng.tensor_mul(
                    pt[:].rearrange("p (a q) -> p a q", q=1024),
                    us[:].unsqueeze(1).broadcast_to((128, 4, 1024)),
                    hidT[hch][lp][:].rearrange("p (a q) -> p a q", q=1024))
                return pt

            def bias_b(n, lp):
                pb = biasps.tile([32, 1024], f32, tag="pb")
                for hf in range(2):
                    r0 = lp * 8 + hf * 4
                    sl = slice(hf * 512, (hf + 1) * 512)
                    lhs0 = w2dr_t[0][:, 1536:1600].rearrange(
                        "p (t m) -> p t m", t=2)
                    lhs1 = w2dr_t[1][:, 1536:1600].rearrange(
                        "p (t m) -> p t m", t=2)
                    nc.tensor.matmul(pb[:, sl], lhs0, win_dr(n, r0, 0, D0),
                                     start=True, stop=False, perf_mode=DR)
                    nc.tensor.matmul(pb[:, sl], lhs1, win_dr(n, r0, O1, D1),
                                     start=False, stop=False, perf_mode=DR)
                    nc.tensor.matmul(pb[:, sl], w2s_t[:, 768:800],
                                     win_s(n, r0, O2),
                                     start=False, stop=True)
                bs = bpool.tile([4, 1024], f16, tag="bs")
                # Relu == identity here: mean (>=103) dominates |bias| (<~2)
                # and the pad row is 0+0
                nc.scalar.activation(bs[:], pb[0:4, :],
                                     mybir.ActivationFunctionType.Relu,
                                     bias=mean4_t[:], scale=USCALE)
                return bs

            def reduce_cc(n, lp, cc, pts, bs):
                for half in range(2):
                    po = outps.tile([128, 512], f32, tag="po")
                    for hch in range(2):
                        for p in range(4):
                            sl = slice(p * 1024 + half * 512,
                                       p * 1024 + half * 512 + 512)
                            nc.tensor.matmul(
                                po[32 * p:32 * p + 32, :],
                                ones_t[:], pts[hch][:, sl],
                                start=(hch == 0), stop=False,
                                skip_group_check=True,
                                tile_position=(0, 32 * p))
                    nc.tensor.matmul(
                        po[:], sel_t[:, cc * 128:(cc + 1) * 128],
                        bs[0:4, half * 512:(half + 1) * 512],
                        start=False, stop=True, skip_group_check=True)
                    posb = bpool.tile([128, 512], f32, tag="posb")
                    if (cc + half) % 2:
                        nc.vector.tensor_copy(posb[:], po[:])
                    else:
                        nc.scalar.activation(
                            posb[:], po[:],
                            mybir.ActivationFunctionType.Copy,
                            bias=0.0, scale=1.0)
                    posrc = posb[:].rearrange("(a b) q -> a b q", b=32)[:, 0, :]
                    nc.sync.dma_start(
                        out[n, cc][:, lp * 1024 + half * 512:
                                   lp * 1024 + half * 512 + 512],
                        posrc)

            # ---- main loop, reduce lagged one cc behind stage B ----
            pending = None
            for n in range(N):
                for lp in range(2):
                    bs = bias_b(n, lp)
                    for cc in range(3):
                        pts = [stage_b(n, lp, cc, hch, cc * 2 + hch)
                               for hch in range(2)]
                        if pending is not None:
                            reduce_cc(*pending)
                        pending = (n, lp, cc, pts, bs)
            reduce_cc(*pending)

    nc.compile()
    return nc


def _host_prep(x, pos_mat, conv_w, conv_b, w1, b1, w2, b2):
    import ml_dtypes
    f = np.float32
    f16 = np.float16
    e4 = ml_dtypes.float8_e4m3

    xpad = np.pad(x, ((0, 0), (0, 0), (3, 3), (3, 3))).astype(f)
    cwr2 = np.zeros((KIM, G0), f)
    cwr2[:75] = conv_w.transpose(1, 2, 3, 0).reshape(75, G0)
    cwr2[75] = -1e4
    cb8 = (FSCALE * conv_b).reshape(G0, 1).astype(f)
    w1a = np.vstack([w1, b1[None, :]]).astype(f)

    Wr = w2.reshape(HH, 576, 3)
    b2r = b2.reshape(576, 3)

    def tap_rows(t):
        return np.concatenate(
            [np.ascontiguousarray(Wr[:, t::9, :].transpose(1, 2, 0)).reshape(G0, 768),
             b2r[t::9, :], np.zeros((G0, 1), f)], axis=1) * WSCALE

    def blk(ta, tb):
        return np.vstack([tap_rows(ta), tap_rows(tb)])

    # DR pair p: [128, 1600] with per-m-block contiguous [ktile0|ktile1]
    # chunks (6 x 256) + a M=32-padded bias block at 1536
    def pack_pair(A, B):
        arr = np.zeros((128, 1600), f)
        for mb in range(6):
            arr[:, mb * 256:mb * 256 + 128] = A[:, mb * 128:(mb + 1) * 128]
            arr[:, mb * 256 + 128:(mb + 1) * 256] = B[:, mb * 128:(mb + 1) * 128]
        arr[:, 1536:1539] = A[:, 768:771]
        arr[:, 1568:1571] = B[:, 768:771]
        return arr

    w2dr = np.stack([pack_pair(blk(0, 1), blk(3, 4)),
                     pack_pair(blk(6, 7), blk(2, 5))]).astype(e4)
    t8 = tap_rows(8)
    w2s = np.zeros((128, 800), f)
    w2s[:G0, :768] = t8[:, :768]
    w2s[:G0, 768:771] = t8[:, 768:771]
    w2s = w2s.astype(e4)

    sel = np.zeros((4, 384), f)
    for cc in range(3):
        sel[cc, cc * 128:(cc + 1) * 128] = 1.0
    mean4 = np.zeros((4, 1), f)
    mean4[:3, 0] = np.asarray(RGB_MEAN, f) * 255.0
    ones16 = np.ones((128, 32), f16)

    from numpy.lib.stride_tricks import sliding_window_view
    in_maps = []
    for core in range(NCORES):
        xsl = xpad[:, :, HS * core: HS * core + HS + 6, :]  # [4,3,22,134]
        sw = sliding_window_view(xsl, (5, 5), axis=(2, 3))  # [4,3,18,130,5,5]
        col = sw.transpose(0, 1, 4, 5, 2, 3).reshape(N, 75, FREE)
        xcol = np.zeros((KIM, N * FREE), f16)
        for n in range(N):
            xcol[:75, n * FREE:(n + 1) * FREE] = col[n]
        ind = np.zeros((FR, FC), f)
        ind[:, 0] = 1.0
        ind[:, FC - 1] = 1.0
        if core == 0:
            ind[0, :] = 1.0
        if core == NCORES - 1:
            ind[FR - 1, :] = 1.0
        xcol[75] = np.tile(ind.reshape(FREE), N)

        pos = pos_mat[0, PR * core: PR * (core + 1), :]
        pos = pos.reshape(2, 8, 2, W, 2, 3).transpose(0, 2, 4, 1, 3, 5).reshape(PR, 3)
        posTc = np.ascontiguousarray(
            np.concatenate([pos, np.ones((PR, 1), f)], 1).T).astype(f16)

        in_maps.append({"xcol": xcol, "posT": posTc,
                        "cwr2": cwr2.astype(f16), "cb8": cb8,
                        "w1a": w1a.astype(f16),
                        "w2dr": w2dr.view(np.uint8),
                        "w2s": w2s.view(np.uint8),
                        "sel": sel.astype(f16), "mean4": mean4,
                        "ones16": ones16})
    return in_maps


def _assemble(results):
    full = np.empty((N, 3, H * SCALE, W * SCALE), np.float32)
    for core in range(NCORES):
        r = results[core]["out"].reshape(N, 3, 2, 2, HS, W)
        blk = r.transpose(0, 1, 4, 2, 5, 3).reshape(N, 3, HS * 2, W * 2)
        full[:, :, HS * 2 * core: HS * 2 * (core + 1), :] = blk
    return full


def kernel(**inputs):
    from concourse.bass_utils import run_bass_kernel_spmd
    if "nc" not in _CACHE:
        _CACHE["nc"] = _build_nc()
    in_maps = _host_prep(**inputs)
    res = run_bass_kernel_spmd(_CACHE["nc"], in_maps, list(range(NCORES)))
    _CACHE["last_result"] = res
    return _assemble(res.results)


# revision 11
# speedup vs baseline: 1.8117x; 1.5743x over previous
"""MetaSR (meta-upscale CNN) Trainium2 kernel, SPMD over 8 NeuronCores.

Algorithm (bilinear reformulation of the reference):
    feat = relu(conv5x5(x) + b)                      [N,64,H,W]
    hid  = relu(pos @ w1 + b1)                       [(H*s*W*s), 256]
    out[n,p,l,c] = sum_h hid[r(p,l),h] * U[n,l,h,c] + bias[n,l,c] + mean_c
      where U[n,l,h,c] = sum_k cols[n,l,k] * w2[h, k*3+c]   (k = 3x3 taps x 64)
            bias[n,l,c] = sum_k cols[n,l,k] * b2[k*3+c]

Sharding: 8 horizontal strips of 16 image rows each (all of N on every core).

v3 pipeline per core:
  - conv as im2col matmul; im2col built on HOST (incl. a -1e4 halo-mask row
    so no on-device fmask multiply), one DMA per n. ACT evicts relu directly
    to fp8e4 (scale 8) into ftb [128, 2*FREE]: cols [0:FREE] = (base | +1col),
    cols [FREE:2FREE] = (base | +1row) via 3 SBUF-SBUF dup DMAs per n.
  - MLP1 from a single posT [4,8192] SBUF tile, interleaved with conv.
  - stage B in fp8e4 DoubleRow: K=576(+pad) as 2 DR matmuls (4 k-tiles) +
    1 plain fp8 matmul, w2 prescaled x16; ACT evicts psum/128 -> us fp16.
  - pt = us (bcast x4) * hidT on DVE (4/6) and GpSimd (2/6).
  - PE reduces over h via ones fp16 matmuls col-group packed (subpixels on
    psum partitions {0,32,64,96}); bias+mean injected via a K=4 sel matmul
    from bs = pb/128 + mean (ACT). Output DMA'd straight from PSUM.
  - reduce(cc) is emitted one stage-B step late so DVE/Pool overlap PE.
"""
import os
import numpy as np

SCALE = 2
RGB_MEAN = (0.4488, 0.4371, 0.404)
N, C, H, W = 4, 3, 128, 128
G0 = 64
NCORES = 8
HS = H // NCORES          # image rows per core (16)
FR = HS + 2               # feat rows incl unfold halo (18)
FC = W + 2                # feat cols incl unfold halo (130)
FREE = FR * FC            # 2340
HH = 256                  # MLP hidden
WCOLS = 3 * HH + 4        # 772 = (c,h) cols + 3 bias cols + 1 pad
KIM = 76                  # im2col rows: 75 conv taps + halo-mask row
LP = HS * W               # pixels per core (2048)
PR = 4 * LP               # pos rows per core (8192)

FSCALE = 8.0              # feat fp8 scale
WSCALE = 16.0             # w2 fp8 scale
USCALE = 1.0 / (FSCALE * WSCALE)

_CACHE = {}


def _build_nc():
    import concourse.bass as bass
    import concourse.tile as tile
    from concourse import bacc, mybir

    f32 = mybir.dt.float32
    f16 = mybir.dt.float16
    f8 = mybir.dt.float8e4
    DR = mybir.MatmulPerfMode.DoubleRow

    nc = bacc.Bacc("TRN2", target_bir_lowering=False, debug=False,
                   num_devices=NCORES)

    xcol = nc.dram_tensor("xcol", [KIM, N * FREE], f16, kind="ExternalInput").ap()
    posT = nc.dram_tensor("posT", [4, PR], f16, kind="ExternalInput").ap()
    cwr2 = nc.dram_tensor("cwr2", [KIM, G0], f16, kind="ExternalInput").ap()
    cb8 = nc.dram_tensor("cb8", [G0, 1], f32, kind="ExternalInput").ap()
    w1a = nc.dram_tensor("w1a", [4, HH], f16, kind="ExternalInput").ap()
    w2dr = nc.dram_tensor("w2dr", [2, 128, 1600], f8, kind="ExternalInput").ap()
    w2s = nc.dram_tensor("w2s", [128, 800], f8, kind="ExternalInput").ap()
    sel = nc.dram_tensor("sel", [4, 384], f16, kind="ExternalInput").ap()
    mean4 = nc.dram_tensor("mean4", [4, 1], f32, kind="ExternalInput").ap()
    ones16 = nc.dram_tensor("ones16", [128, 32], f16, kind="ExternalInput").ap()
    out = nc.dram_tensor("out", [N, 3, 4, LP], f32, kind="ExternalOutput").ap()

    with tile.TileContext(nc) as tc:
        with tc.tile_pool(name="const", bufs=1) as cpool, \
             tc.tile_pool(name="feat", bufs=1) as fpool, \
             tc.tile_pool(name="hid", bufs=1) as hpool, \
             tc.tile_pool(name="im2col", bufs=2) as xpool, \
             tc.tile_pool(name="usb", bufs=3) as upool, \
             tc.tile_pool(name="pt", bufs=4) as ppool, \
             tc.tile_pool(name="bsb", bufs=2) as bpool, \
             tc.tile_pool(name="ups", bufs=2, space="PSUM") as ups, \
             tc.tile_pool(name="biasps", bufs=1, space="PSUM") as biasps, \
             tc.tile_pool(name="outps", bufs=2, space="PSUM") as outps:

            # ---- constants to SBUF; conv-critical ones first on sync ----
            cwr2_t = cpool.tile([KIM, G0], f16, tag="cwr2")
            nc.sync.dma_start(cwr2_t[:], cwr2[:])
            cb8_t = cpool.tile([G0, 1], f32, tag="cb8")
            nc.sync.dma_start(cb8_t[:], cb8[:])
            xt0 = xpool.tile([KIM, FREE], f16, tag="x")
            nc.sync.dma_start(
                xt0[:], bass.AP(xcol.tensor, 0, [[N * FREE, KIM], [1, FREE]]))
            # off-critical consts on other queues
            posT_t = cpool.tile([4, PR], f16, tag="posT")
            nc.gpsimd.dma_start(posT_t[:], posT[:])
            w1a_t = cpool.tile([4, HH], f16, tag="w1a")
            nc.gpsimd.dma_start(w1a_t[:], w1a[:])
            w2dr_t = []
            for p in range(2):
                t = cpool.tile([128, 1600], f8, tag=f"w2dr{p}")
                nc.scalar.dma_start(t[:], w2dr[p])
                w2dr_t.append(t)
            w2s_t = cpool.tile([128, 800], f8, tag="w2s")
            nc.scalar.dma_start(w2s_t[:], w2s[:])
            sel_t = cpool.tile([4, 384], f16, tag="sel")
            nc.scalar.dma_start(sel_t[:], sel[:])
            mean4_t = cpool.tile([4, 1], f32, tag="mean4")
            nc.scalar.dma_start(mean4_t[:], mean4[:])
            ones_t = cpool.tile([128, 32], f16, tag="ones16")
            nc.scalar.dma_start(ones_t[:], ones16[:])

            ftb = [fpool.tile([128, 2 * FREE], f8, tag=f"ftb{n}",
                              name=f"ftb{n}")
                   for n in range(N)]
            hidT = [[None] * 2, [None] * 2]

            def conv_n(n, xt):
                ft = ftb[n]
                for ch in range(5):
                    lo = ch * 512
                    hi = min(FREE, lo + 512)
                    ps = outps.tile([128, 512], f32, tag="po")
                    nc.tensor.matmul(ps[0:G0, : hi - lo], cwr2_t[:],
                                     xt[:, lo:hi], start=True, stop=True)
                    nc.scalar.activation(ft[0:G0, lo:hi], ps[0:G0, : hi - lo],
                                         mybir.ActivationFunctionType.Relu,
                                         bias=cb8_t[:], scale=FSCALE)
                u8 = mybir.dt.uint8
                # upper half of cols [0:FREE] = +1 element (col) shift
                nc.scalar.dma_start(ft[G0:128, 0:FREE - 1], ft[0:G0, 1:FREE])
                nc.vector.memset(ft[G0:128, FREE - 1:FREE].bitcast(u8), 0)
                # cols [FREE:2FREE] = (base | +1 row)
                nc.gpsimd.dma_start(ft[0:G0, FREE:2 * FREE], ft[0:G0, 0:FREE])
                nc.gpsimd.dma_start(ft[G0:128, FREE:2 * FREE - FC],
                                    ft[0:G0, FC:FREE])
                nc.vector.memset(
                    ft[G0:128, 2 * FREE - FC:2 * FREE].bitcast(u8), 0)

            def mlp1_tile(hch, lp):
                hb = hpool.tile([128, 4096], f16, tag=f"hid{hch}_{lp}")
                for pair in range(4):
                    ps = ups.tile([128, 1024], f32, tag="pu")
                    for sub in range(2):
                        base = lp * 4096 + pair * 1024 + sub * 512
                        nc.tensor.matmul(ps[:, sub * 512:(sub + 1) * 512],
                                         w1a_t[:, hch * 128:(hch + 1) * 128],
                                         posT_t[:, base:base + 512],
                                         start=True, stop=True)
                    nc.scalar.activation(
                        hb[:, pair * 1024:(pair + 1) * 1024], ps[:],
                        mybir.ActivationFunctionType.Relu, bias=0.0, scale=1.0)
                hidT[hch][lp] = hb

            # ---- conv + MLP1 interleaved ----
            mlp_order = [(0, 0), (0, 1), (1, 0), (1, 1)]
            for n in range(N):
                xt = xt0 if n == 0 else xpool.tile([KIM, FREE], f16, tag="x")
                if n > 0:
                    nc.sync.dma_start(
                        xt[:], bass.AP(xcol.tensor, n * FREE,
                                       [[N * FREE, KIM], [1, FREE]]))
                conv_n(n, xt)
                mlp1_tile(*mlp_order[n])

            # window AP into ftb[n]: k-tile pair (DoubleRow rhs) or single
            def win_dr(n, r0, off0, delta):
                ap = ftb[n][:]
                return bass.AP(ap.tensor, ap.offset + r0 * FC + off0,
                               [[2 * FREE, 128], [delta, 2], [FC, 4], [1, W]])

            def win_s(n, r0, off0):
                ap = ftb[n][:]
                return bass.AP(ap.tensor, ap.offset + r0 * FC + off0,
                               [[2 * FREE, 128], [FC, 4], [1, W]])

            D0 = FC                    # pair0: taps(0,1)@(r0,0) / (3,4)@(r0+1,0)
            O1 = 2 * FC                # pair1 ktile0: taps(6,7)@(r0+2,0)
            D1 = FREE - 2 * FC + 2     # pair1 ktile1: taps(2,5)@fb(r0,2)
            O2 = 2 * FC + 2            # single: tap8@(r0+2,2)

            def stage_b(n, lp, cc, hch, ptidx):
                mb = cc * 2 + hch
                pu = ups.tile([128, 1024], f32, tag="pu")
                for hf in range(2):
                    r0 = lp * 8 + hf * 4
                    sl = slice(hf * 512, (hf + 1) * 512)
                    lhs0 = w2dr_t[0][:, mb * 256:(mb + 1) * 256].rearrange(
                        "p (t m) -> p t m", t=2)
                    lhs1 = w2dr_t[1][:, mb * 256:(mb + 1) * 256].rearrange(
                        "p (t m) -> p t m", t=2)
                    nc.tensor.matmul(pu[:, sl], lhs0, win_dr(n, r0, 0, D0),
                                     start=True, stop=False, perf_mode=DR)
                    nc.tensor.matmul(pu[:, sl], lhs1, win_dr(n, r0, O1, D1),
                                     start=False, stop=False, perf_mode=DR)
                    nc.tensor.matmul(pu[:, sl],
                                     w2s_t[:, mb * 128:(mb + 1) * 128],
                                     win_s(n, r0, O2),
                                     start=False, stop=True)
                us = upool.tile([128, 1024], f16, tag="us")
                nc.scalar.activation(us[:], pu[:],
                                     mybir.ActivationFunctionType.Copy,
                                     bias=0.0, scale=USCALE)
                pt = ppool.tile([128, 4096], f16, tag="pt")
                nc.vector.tensor_mul(
                    pt[:].rearrange("p (a q) -> p a q", q=1024),
                    us[:].unsqueeze(1).broadcast_to((128, 4, 1024)),
                    hidT[hch][lp][:].rearrange("p (a q) -> p a q", q=1024))
                return pt

            def bias_b(n, lp):
                pb = biasps.tile([32, 1024], f32, tag="pb")
                for hf in range(2):
                    r0 = lp * 8 + hf * 4
                    sl = slice(hf * 512, (hf + 1) * 512)
                    lhs0 = w2dr_t[0][:, 1536:1600].rearrange(
                        "p (t m) -> p t m", t=2)
                    lhs1 = w2dr_t[1][:, 1536:1600].rearrange(
                        "p (t m) -> p t m", t=2)
                    nc.tensor.matmul(pb[:, sl], lhs0, win_dr(n, r0, 0, D0),
                                     start=True, stop=False, perf_mode=DR)
                    nc.tensor.matmul(pb[:, sl], lhs1, win_dr(n, r0, O1, D1),
                                     start=False, stop=False, perf_mode=DR)
                    nc.tensor.matmul(pb[:, sl], w2s_t[:, 768:800],
                                     win_s(n, r0, O2),
                                     start=False, stop=True)
                bs = bpool.tile([4, 1024], f16, tag="bs")
                # Relu == identity here: mean (>=103) dominates |bias| (<~2)
                # and the pad row is 0+0
                nc.scalar.activation(bs[:], pb[0:4, :],
                                     mybir.ActivationFunctionType.Relu,
                                     bias=mean4_t[:], scale=USCALE)
                return bs

            def reduce_cc(n, lp, cc, pts, bs):
                for half in range(2):
                    po = outps.tile([128, 512], f32, tag="po")
                    for hch in range(2):
                        for p in range(4):
                            sl = slice(p * 1024 + half * 512,
                                       p * 1024 + half * 512 + 512)
                            nc.tensor.matmul(
                                po[32 * p:32 * p + 32, :],
                                ones_t[:], pts[hch][:, sl],
                                start=(hch == 0), stop=False,
                                skip_group_check=True,
                                tile_position=(0, 32 * p))
                    nc.tensor.matmul(
                        po[:], sel_t[:, cc * 128:(cc + 1) * 128],
                        bs[0:4, half * 512:(half + 1) * 512],
                        start=False, stop=True, skip_group_check=True)
                    posb = bpool.tile([128, 512], f32, tag="posb")
                    nc.scalar.activation(
                        posb[:], po[:],
                        mybir.ActivationFunctionType.Copy,
                        bias=0.0, scale=1.0)
                    posrc = posb[:].rearrange("(a b) q -> a b q", b=32)[:, 0, :]
                    nc.sync.dma_start(
                        out[n, cc][:, lp * 1024 + half * 512:
                                   lp * 1024 + half * 512 + 512],
                        posrc)

            # ---- main loop, reduce lagged one cc behind stage B ----
            pending = None
            for n in range(N):
                for lp in range(2):
                    bs = bias_b(n, lp)
                    for cc in range(3):
                        pts = [stage_b(n, lp, cc, hch, cc * 2 + hch)
                               for hch in range(2)]
                        if pending is not None:
                            reduce_cc(*pending)
                        pending = (n, lp, cc, pts, bs)
            reduce_cc(*pending)

    nc.compile()
    return nc


def _host_prep(x, pos_mat, conv_w, conv_b, w1, b1, w2, b2):
    import ml_dtypes
    f = np.float32
    f16 = np.float16
    e4 = ml_dtypes.float8_e4m3

    xpad = np.pad(x, ((0, 0), (0, 0), (3, 3), (3, 3))).astype(f)
    cwr2 = np.zeros((KIM, G0), f)
    cwr2[:75] = conv_w.transpose(1, 2, 3, 0).reshape(75, G0)
    cwr2[75] = -1e4
    cb8 = (FSCALE * conv_b).reshape(G0, 1).astype(f)
    w1a = np.vstack([w1, b1[None, :]]).astype(f)

    Wr = w2.reshape(HH, 576, 3)
    b2r = b2.reshape(576, 3)

    def tap_rows(t):
        return np.concatenate(
            [np.ascontiguousarray(Wr[:, t::9, :].transpose(1, 2, 0)).reshape(G0, 768),
             b2r[t::9, :], np.zeros((G0, 1), f)], axis=1) * WSCALE

    def blk(ta, tb):
        return np.vstack([tap_rows(ta), tap_rows(tb)])

    # DR pair p: [128, 1600] with per-m-block contiguous [ktile0|ktile1]
    # chunks (6 x 256) + a M=32-padded bias block at 1536
    def pack_pair(A, B):
        arr = np.zeros((128, 1600), f)
        for mb in range(6):
            arr[:, mb * 256:mb * 256 + 128] = A[:, mb * 128:(mb + 1) * 128]
            arr[:, mb * 256 + 128:(mb + 1) * 256] = B[:, mb * 128:(mb + 1) * 128]
        arr[:, 1536:1539] = A[:, 768:771]
        arr[:, 1568:1571] = B[:, 768:771]
        return arr

    w2dr = np.stack([pack_pair(blk(0, 1), blk(3, 4)),
                     pack_pair(blk(6, 7), blk(2, 5))]).astype(e4)
    t8 = tap_rows(8)
    w2s = np.zeros((128, 800), f)
    w2s[:G0, :768] = t8[:, :768]
    w2s[:G0, 768:771] = t8[:, 768:771]
    w2s = w2s.astype(e4)

    sel = np.zeros((4, 384), f)
    for cc in range(3):
        sel[cc, cc * 128:(cc + 1) * 128] = 1.0
    mean4 = np.zeros((4, 1), f)
    mean4[:3, 0] = np.asarray(RGB_MEAN, f) * 255.0
    ones16 = np.ones((128, 32), f16)

    from numpy.lib.stride_tricks import sliding_window_view
    in_maps = []
    for core in range(NCORES):
        xsl = xpad[:, :, HS * core: HS * core + HS + 6, :]  # [4,3,22,134]
        sw = sliding_window_view(xsl, (5, 5), axis=(2, 3))  # [4,3,18,130,5,5]
        col = sw.transpose(0, 1, 4, 5, 2, 3).reshape(N, 75, FREE)
        xcol = np.zeros((KIM, N * FREE), f16)
        for n in range(N):
            xcol[:75, n * FREE:(n + 1) * FREE] = col[n]
        ind = np.zeros((FR, FC), f)
        ind[:, 0] = 1.0
        ind[:, FC - 1] = 1.0
        if core == 0:
            ind[0, :] = 1.0
        if core == NCORES - 1:
            ind[FR - 1, :] = 1.0
        xcol[75] = np.tile(ind.reshape(FREE), N)

        pos = pos_mat[0, PR * core: PR * (core + 1), :]
        pos = pos.reshape(2, 8, 2, W, 2, 3).transpose(0, 2, 4, 1, 3, 5).reshape(PR, 3)
        posTc = np.ascontiguousarray(
            np.concatenate([pos, np.ones((PR, 1), f)], 1).T).astype(f16)

        in_maps.append({"xcol": xcol, "posT": posTc,
                        "cwr2": cwr2.astype(f16), "cb8": cb8,
                        "w1a": w1a.astype(f16),
                        "w2dr": w2dr.view(np.uint8),
                        "w2s": w2s.view(np.uint8),
                        "sel": sel.astype(f16), "mean4": mean4,
                        "ones16": ones16})
    return in_maps


def _assemble(results):
    full = np.empty((N, 3, H * SCALE, W * SCALE), np.float32)
    for core in range(NCORES):
        r = results[core]["out"].reshape(N, 3, 2, 2, HS, W)
        blk = r.transpose(0, 1, 4, 2, 5, 3).reshape(N, 3, HS * 2, W * 2)
        full[:, :, HS * 2 * core: HS * 2 * (core + 1), :] = blk
    return full


def kernel(**inputs):
    from concourse.bass_utils import run_bass_kernel_spmd
    if "nc" not in _CACHE:
        _CACHE["nc"] = _build_nc()
    in_maps = _host_prep(**inputs)
    res = run_bass_kernel_spmd(_CACHE["nc"], in_maps, list(range(NCORES)))
    _CACHE["last_result"] = res
    return _assemble(res.results)
